# revision 2
# baseline (speedup 1.0000x reference)
"""Cantor global attention kernel for Trainium2 (8 NeuronCores, SPMD), v5.

Data-parallel over batch B=64 -> 8 cores x 8 rows each.  Per core the
full expert axis lives in the SBUF free dimension: partition = (b, ph)
= 128, free = (expert-position, 256 cols).  All element-wise compute
runs in fp16 so DVE tensor_tensor hits the 2x_1P perf mode (fp32 TT is
1x) -- the fp32 baseline's 62us of DVE work halves.

Structure (all engines pipelined by the Tile scheduler):
  - loads:   SWDGE (gpsimd) DMA with f32->fp16 cast, one DMA per
             (tensor, expert-chunk) covering both projections.  The
             big route-cluster loads first chunk-by-chunk (k then q,
             interleaved) so DVE starts ~11us in; v strictly last
             (only prods need it), small cluster's v at the very end
             so the post-load tail is one small chunk deep.
  - avg:     DVE fp16 adds (projection sum; /2 scalings fold into the
             exp coefficient and the ln scale).
  - t = q*k: DVE fp16 TT, batched over runs of experts whose route
             offset is locally constant (host-optimized expert order +
             per-expert route-slot assignment minimize run count).
  - exp:     ScalarE activation, scale = per-(e,w) coefficient,
             cluster-wide groups batched over equal immediates.  A
             host-side fp16 range emulation proves the data needs no
             overflow shift and no den-underflow guard (the fallback
             path adds per-expert bias shifts + a ln epsilon).  The
             one ACT table set holding BOTH exp and ln is pinned up
             front (the auto-inserter would thrash 5 table loads).
  - prod:    DVE fp16 TT against the gathered V slabs.
  - sums:    two DVE adds over a combined [den|num] layout.
  - 1/den:   ScalarE r = exp(-ln(2*den)) -- the 0.5 output scaling
             rides the ln scale immediate for free.
  - out:     DVE fp16 mul, then SWDGE cast store fp16->f32.
  Everything from prod down runs in expert-chunks (3+3+4 | 3+3) so
  compute unlocks piecewise behind the loads.

The expert order in SBUF is a host-chosen permutation; the host
unshards/unpermutes the output slabs when assembling the full result.
"""

import math

import numpy as np

import concourse.bass as bass
import concourse.mybir as mybir
from concourse import bacc, tile
from concourse.bass_utils import run_bass_kernel_spmd

E, NPROJ, B, P = 16, 2, 64, 4096
W = 3
EXPERT_DIM = 128
NCORES = 8
BS = B // NCORES          # 8 batch rows per core
COLS = 256                # free-dim columns per expert slab
PH = P // COLS            # 16 partition sub-blocks per batch row
PART = BS * PH            # 128 SBUF partitions
EC = E * COLS             # 4096 free-dim columns total

F16 = mybir.dt.float16
F32 = mybir.dt.float32
EXPF = mybir.ActivationFunctionType.Exp
LNF = mybir.ActivationFunctionType.Ln

EXP_CLAMP = 8.2           # fallback-path max exp argument after shift
DEN_EPS = 3e-5            # fallback-path ln(den + eps) underflow guard
FP16_SAFE_MAX = 45000.0   # host-emulation bound for the no-shift path
FP16_SAFE_MINDEN = 1e-6


# ---------------------------------------------------------------- planning

def _route_components(routes):
    """Connected components of the expert route graph, ordered largest
    component first (its q lands early, so its larger exp workload
    starts early; the small component forms the short post-load tail)."""
    parent = list(range(E))

    def find(x):
        while parent[x] != x:
            parent[x] = parent[parent[x]]
            x = parent[x]
        return x

    for e in range(E):
        for j in routes[e]:
            a, b = find(e), find(int(j))
            if a != b:
                parent[a] = b
    comps = {}
    for e in range(E):
        comps.setdefault(find(e), []).append(e)
    return sorted(comps.values(), key=lambda c: (-len(c), min(c)))


def _slot_dp(edges):
    """Assign each position's 3 target-positions to 3 route slots,
    minimizing the number of maximal diagonal runs per slot.

    edges[p] = list of 3 (tpos, eidx) pairs.  Returns (runcount,
    slots) with slots[p] = tuple of 3 (tpos, eidx) in slot order."""
    import itertools

    n = len(edges)
    states = {(-1, -1, -1): (0, None, None)}  # state -> (cost, prev, pairing)
    layers = []
    for p in range(n):
        layers.append(states)
        nxt = {}
        for st, (cost, _, _) in states.items():
            for perm in itertools.permutations(range(3)):
                c = cost + sum(
                    1 for s in range(3) if edges[p][perm[s]][0] != st[s])
                ns = tuple(edges[p][perm[s]][0] + 1 for s in range(3))
                if ns not in nxt or nxt[ns][0] > c:
                    nxt[ns] = (c, st, perm)
        states = nxt
    cur = min(states, key=lambda k: states[k][0])
    total = states[cur][0]
    slots = [None] * n
    curmap = states
    for p in range(n - 1, -1, -1):
        _, prev, perm = curmap[cur]
        slots[p] = tuple(edges[p][perm[s]] for s in range(3))
        cur, curmap = prev, layers[p]
    return total, slots


def _optimize_component(comp, routes, iters=300, seed=0):
    """Pick an order of the component's experts + per-expert slot
    assignment minimizing DVE run count.  Returns (order, slots)."""
    rng = np.random.RandomState(seed)
    targets = {e: [int(j) for j in routes[e]] for e in comp}

    def evaluate(order):
        pos = {e: i for i, e in enumerate(order)}
        edges = [[(pos[j], wi) for wi, j in enumerate(targets[e])]
                 for e in order]
        return _slot_dp(edges)

    best_order = list(comp)
    best_cost, best_slots = evaluate(best_order)
    order = list(best_order)
    cost = best_cost
    for it in range(iters):
        a, b = rng.randint(len(order)), rng.randint(len(order))
        if a == b:
            continue
        cand = list(order)
        cand[a], cand[b] = cand[b], cand[a]
        c, s = evaluate(cand)
        if c <= cost:
            order, cost = cand, c
            if c < best_cost:
                best_order, best_cost, best_slots = cand, c, s
    return best_order, best_slots


def _runs_from_slots(slots, pbase):
    runs = [[] for _ in range(W)]
    for w in range(W):
        for p, sl in enumerate(slots):
            t, _ = sl[w]
            if (runs[w] and runs[w][-1][0] + runs[w][-1][2] == p + pbase
                    and runs[w][-1][1] + runs[w][-1][2] == t + pbase):
                runs[w][-1][2] += 1
            else:
                runs[w].append([p + pbase, t + pbase, 1])
    return runs


def _clip_runs(runs, lo, hi):
    out = []
    for w in range(W):
        for p0, t0, L in runs[w]:
            a, b = max(p0, lo), min(p0 + L, hi)
            if a < b:
                out.append((w, a, t0 + (a - p0), b - a))
    return out


def _chunk_sizes(n):
    """Load-chunk split: a small first chunk starts compute early."""
    if n >= 7:
        return [2, (n - 2 + 1) // 2, (n - 2) // 2]
    if n >= 4:
        a = n // 2
        return [a, n - a]
    return [n]


def _tail_sizes(n):
    """Tail-chunk split for prods/sums/normalize/out/store."""
    if n >= 7:
        a = (n + 2) // 3
        b = (n - a + 1) // 2
        return [a, b, n - a - b]
    if n >= 4:
        a = n // 2
        return [a, n - a]
    return [n]


def _bounds(pbase, sizes):
    b = [pbase]
    for s in sizes:
        b.append(b[-1] + s)
    return list(zip(b[:-1], b[1:]))


def make_plan(routes, betas, temperature, shifts, use_eps):
    """Host control plane.  shifts[e] = softmax-invariant exp-argument
    shift (>=0, all zero on the fast path).  Returns a hashable plan."""
    scale = np.float32(math.sqrt(EXPERT_DIM)) * abs(np.float32(temperature))
    gate = np.where(routes != np.arange(E, dtype=routes.dtype)[:, None],
                    1.0 / (1.0 + np.exp(-betas.astype(np.float64))),
                    1.0)
    coef = (0.25 * gate / scale).astype(np.float32)

    comps = _route_components(routes)
    perm = []
    comp_plans = []
    for ci, comp in enumerate(comps):
        order, slots = _optimize_component(comp, routes, seed=ci)
        pbase = len(perm)
        n = len(comp)
        lchunks = _bounds(pbase, _chunk_sizes(n))       # load/avg chunks
        tchunks = _bounds(pbase, _tail_sizes(n))        # tail chunks
        # t-runs clipped coarsely (halves) to bound instruction count
        mid = pbase + (n + 1) // 2
        hsplit = [(pbase, mid), (mid, pbase + n)] if n >= 6 \
            else [(pbase, pbase + n)]
        runs = _runs_from_slots(slots, pbase)
        # component-wide exp groups (consecutive positions in a slot
        # with equal scale/bias immediates)
        egroups = []
        for w in range(W):
            for p in range(pbase, pbase + n):
                e = order[p - pbase]
                cval = float(coef[e, slots[p - pbase][w][1]])
                bval = -float(shifts[e])
                if (egroups and egroups[-1][0] == w
                        and egroups[-1][1] + egroups[-1][2] == p
                        and egroups[-1][3] == cval and egroups[-1][4] == bval):
                    egroups[-1][2] += 1
                else:
                    egroups.append([w, p, 1, cval, bval])
        comp_plans.append((
            pbase, n,
            tuple(lchunks),
            tuple(tchunks),
            tuple(tuple(_clip_runs(runs, lo, hi)) for lo, hi in hsplit),
            tuple(tuple(_clip_runs(runs, lo, hi)) for lo, hi in tchunks),
            tuple(tuple(g) for g in egroups),
        ))
        perm.extend(order)
    return tuple(perm), tuple(comp_plans), bool(use_eps)


# ----------------------------------------------------------------- builder

def _register_bias_consts(nc, values):
    """Materialize activation-bias constants as [128,1] SBUF const APs
    (mirrors Bass.__init__'s register_const_ap).  No-op when every bias
    is an already-registered value (the fast path)."""
    fresh = False
    for v in sorted(set(float(v) for v in values)):
        if (F32, v) in nc.const_aps.aps:
            continue
        t = nc.alloc_sbuf_tensor(f"const-f32-{v}", [128, 1], F32)
        nc.gpsimd.memset(t.ap(), v)
        nc.const_aps.aps[(F32, v)] = t.ap()
        fresh = True
    if fresh:
        nc.all_engine_barrier()


def _build_nc(plan):
    perm, comp_plans, use_eps = plan
    nc = bacc.Bacc("TRN2", target_bir_lowering=False, debug=False,
                   num_devices=NCORES)

    ln_bias = float(DEN_EPS) if use_eps else 0.0
    _register_bias_consts(
        nc,
        [2.0 * ln_bias]
        + [g[4] for cp in comp_plans for g in cp[6]])

    # Pin the one ACT table set containing BOTH exp and ln
    # (natural_log_exp_and_others): the auto-inserter first-matches
    # exp -> exp_and_others and ln -> natural_log, reloading tables on
    # every exp/ln alternation (5 loads, ~6.4us, serializing ScalarE).
    from concourse.hw_specs import get_activation_tables
    tables = list(get_activation_tables(nc.m.arch).items())
    combined_id = next(
        i for i, (name, funcs) in enumerate(tables)
        if mybir.ActivationFunctionType.Exp in funcs
        and mybir.ActivationFunctionType.Ln in funcs)
    nc.scalar.add_instruction(mybir.InstLoadActFuncSet(
        name=nc.get_next_instruction_name(),
        act_func_set_id=combined_id))

    q_d = nc.dram_tensor("q", [PART, NPROJ * EC], F32, kind="ExternalInput")
    k_d = nc.dram_tensor("k", [PART, NPROJ * EC], F32, kind="ExternalInput")
    v_d = nc.dram_tensor("v", [PART, NPROJ * EC], F32, kind="ExternalInput")
    o_d = nc.dram_tensor("out", [PART, EC], F32, kind="ExternalOutput")

    qv = q_d.ap().rearrange("p (n c) -> p n c", n=NPROJ)
    kv = k_d.ap().rearrange("p (n c) -> p n c", n=NPROJ)
    vv = v_d.ap().rearrange("p (n c) -> p n c", n=NPROJ)

    with tile.TileContext(nc) as tc:
        with (
            tc.tile_pool(name="raw", bufs=1) as raw_p,
            tc.tile_pool(name="sum", bufs=1) as sum_p,
            tc.tile_pool(name="tp", bufs=1) as tp_p,
            tc.tile_pool(name="dn", bufs=1) as dn_p,
            tc.tile_pool(name="og", bufs=1) as og_p,
        ):
            raw_q = raw_p.tile([PART, NPROJ * EC], F16, name="rq", tag="rq")
            raw_k = raw_p.tile([PART, NPROJ * EC], F16, name="rk", tag="rk")
            raw_v = raw_p.tile([PART, NPROJ * EC], F16, name="rv", tag="rv")
            qs = sum_p.tile([PART, EC], F16, name="qs", tag="qs")
            ks = sum_p.tile([PART, EC], F16, name="ks", tag="ks")
            vs = sum_p.tile([PART, EC], F16, name="vs", tag="vs")
            tp = tp_p.tile([PART, W * 2 * EC], F16, name="tp", tag="tp")
            dn = dn_p.tile([PART, 2 * EC], F16, name="dn", tag="dn")
            og = og_p.tile([PART, EC], F16, name="og", tag="og")

            rqv = raw_q[:].rearrange("p (n c) -> p n c", n=NPROJ)
            rkv = raw_k[:].rearrange("p (n c) -> p n c", n=NPROJ)
            rvv = raw_v[:].rearrange("p (n c) -> p n c", n=NPROJ)
            tpv = tp[:].rearrange("p (w k c) -> p w k c", w=W, k=2)
            dnv = dn[:].rearrange("p (k c) -> p k c", k=2)

            def csl(lo, hi):
                return slice(lo * COLS, hi * COLS)

            def load(raw, drv, lo, hi):
                nc.gpsimd.dma_start(raw[:, :, csl(lo, hi)],
                                    drv[:, :, csl(lo, hi)])

            # ---- loads (SWDGE cast f32->fp16; the in-order SWDGE queue
            # drains in emission order = bandwidth priority).  k/q chunk-
            # interleaved per cluster, big cluster first; all v last,
            # small cluster's v at the very end.  The very first chunk's
            # k/q loads split per-projection: more DMAs queued early ->
            # the SDMA pipeline reaches steady-state rate sooner.
            first = True
            for cp in comp_plans:
                for lo, hi in cp[2]:
                    if first:
                        for rv_, dv_ in ((rkv, kv), (rqv, qv)):
                            for np_ in range(NPROJ):
                                nc.gpsimd.dma_start(
                                    rv_[:, np_, csl(lo, hi)],
                                    dv_[:, np_, csl(lo, hi)])
                        first = False
                    else:
                        load(rkv, kv, lo, hi)
                        load(rqv, qv, lo, hi)
            for cp in comp_plans:
                for lo, hi in cp[3]:
                    load(rvv, vv, lo, hi)

            # ---- compute, cluster-major
            for (pbase, n, lchunks, tchunks, hruns, truns,
                 egroups) in comp_plans:
                # projection sums per load chunk (k first)
                for lo, hi in lchunks:
                    sl = csl(lo, hi)
                    nc.vector.tensor_add(ks[:, sl], rkv[:, 0, sl],
                                         rkv[:, 1, sl])
                    nc.vector.tensor_add(qs[:, sl], rqv[:, 0, sl],
                                         rqv[:, 1, sl])

                # t = qs * ks[route], runs clipped at half boundaries
                for runs in hruns:
                    for w, p0, t0, L in runs:
                        nc.vector.tensor_mul(
                            tpv[:, w, 0, csl(p0, p0 + L)],
                            qs[:, csl(p0, p0 + L)],
                            ks[:, csl(t0, t0 + L)])

                # e_w = exp(coef * t - shift), cluster-wide groups
                for w, p0, L, cval, bval in egroups:
                    psl = csl(p0, p0 + L)
                    nc.scalar.activation(tpv[:, w, 0, psl],
                                         tpv[:, w, 0, psl],
                                         EXPF, bias=bval, scale=cval)

                # v sums per tail chunk (prods consume them per chunk)
                for lo, hi in tchunks:
                    sl = csl(lo, hi)
                    nc.vector.tensor_add(vs[:, sl], rvv[:, 0, sl],
                                         rvv[:, 1, sl])

                # per-chunk tail: prods, sums, normalize, out, store
                for (lo, hi), runs in zip(tchunks, truns):
                    sl = csl(lo, hi)
                    for w, p0, t0, L in runs:
                        nc.vector.tensor_mul(
                            tpv[:, w, 1, csl(p0, p0 + L)],
                            tpv[:, w, 0, csl(p0, p0 + L)],
                            vs[:, csl(t0, t0 + L)])
                    nc.vector.tensor_add(dnv[:, :, sl], tpv[:, 0, :, sl],
                                         tpv[:, 1, :, sl])
                    nc.vector.tensor_add(dnv[:, :, sl], dnv[:, :, sl],
                                         tpv[:, 2, :, sl])
                    # r = 0.5/den = exp(-ln(2*den [+2eps]))
                    nc.scalar.activation(dnv[:, 0, sl], dnv[:, 0, sl], LNF,
                                         bias=2.0 * ln_bias, scale=2.0)
                    nc.scalar.activation(dnv[:, 0, sl], dnv[:, 0, sl], EXPF,
                                         bias=0.0, scale=-1.0)
                    nc.vector.tensor_mul(og[:, sl], dnv[:, 1, sl],
                                         dnv[:, 0, sl])
                    nc.gpsimd.dma_start(o_d.ap()[:, sl], og[:, sl])

    nc.compile()
    return nc


_cache: dict = {}


def _get_nc(plan):
    if plan not in _cache:
        _cache[plan] = _build_nc(plan)
    return _cache[plan]


# ------------------------------------------------------------------ driver

def _stage_inputs(Q, K, V, perm):
    """Per-core host staging: [E,2,BS,P] -> [PART, NPROJ, pos, COLS] f32."""
    maps = []
    permv = np.asarray(perm, dtype=np.int64)
    for c in range(NCORES):
        def lay(t):
            x = t[permv][:, :, c * BS:(c + 1) * BS, :]       # [E,2,BS,P]
            x = x.reshape(E, NPROJ, BS, PH, COLS)
            x = x.transpose(2, 3, 1, 0, 4)                   # b ph n pos c
            return np.ascontiguousarray(
                x.reshape(PART, NPROJ * EC), dtype=np.float32)
        maps.append({"q": lay(Q), "k": lay(K), "v": lay(V)})
    return maps


def _unstage_output(res, perm):
    permv = np.asarray(perm, dtype=np.int64)
    out = np.empty((B, E * P), dtype=np.float32)
    for c in range(NCORES):
        arr = res[c]["out"].reshape(BS, PH, E, COLS)         # b ph pos c
        full = np.empty((BS, E, PH, COLS), dtype=np.float32)
        full[:, permv] = arr.transpose(0, 2, 1, 3)           # b pos ph c
        out[c * BS:(c + 1) * BS] = full.reshape(BS, E * P)
    return out


def _host_ranges(Q, K, V, routes, betas, temperature):
    """fp16 emulation of the device pipeline's value ranges.  Returns
    (shifts, use_eps): all-zero shifts + no eps when provably safe."""
    scale = np.float32(math.sqrt(EXPERT_DIM)) * abs(np.float32(temperature))
    gate = np.where(routes != np.arange(E, dtype=routes.dtype)[:, None],
                    1.0 / (1.0 + np.exp(-betas.astype(np.float64))),
                    1.0)
    coef = (0.25 * gate / scale).astype(np.float32)
    qs = Q.astype(np.float16).sum(axis=1, dtype=np.float16)
    ks = K.astype(np.float16).sum(axis=1, dtype=np.float16)
    vs = V.astype(np.float16).sum(axis=1, dtype=np.float16)
    maxnum = 0.0
    maxden = 0.0
    minden = np.inf
    maxargs = np.zeros(E, dtype=np.float64)
    for e in range(E):
        t = (qs[e][None] * ks[routes[e]]).astype(np.float16)
        arg = (coef[e][:, None, None].astype(np.float32)
               * t.astype(np.float32))
        maxargs[e] = arg.max()
        ew = np.exp(np.minimum(arg, 11.0)).astype(np.float16)
        prod = (ew * vs[routes[e]]).astype(np.float16)
        num = ((prod[0] + prod[1]).astype(np.float16)
               + prod[2]).astype(np.float16)
        den = ((ew[0] + ew[1]).astype(np.float16)
               + ew[2]).astype(np.float16)
        maxnum = max(maxnum, float(np.abs(num.astype(np.float32)).max()))
        maxden = max(maxden, float(den.astype(np.float32).max()))
        minden = min(minden, float(den.astype(np.float32).min()))
    safe = (maxargs.max() < 10.0 and maxnum < FP16_SAFE_MAX
            and maxden < FP16_SAFE_MAX and minden > FP16_SAFE_MINDEN)
    if safe:
        return tuple(0.0 for _ in range(E)), False
    shifts = np.maximum(0.0, maxargs - EXP_CLAMP)
    return tuple(float(np.float32(round(s * 4) / 4.0)) for s in shifts), True


def prepare(Q_proj, K_proj, V_proj, betas, temperature, routes, num_patches):
    Q = np.asarray(Q_proj, dtype=np.float32)
    K = np.asarray(K_proj, dtype=np.float32)
    V = np.asarray(V_proj, dtype=np.float32)
    betas = np.asarray(betas, dtype=np.float32)
    temp = float(np.asarray(temperature, dtype=np.float32).reshape(-1)[0])
    routes = np.asarray(routes, dtype=np.int32)
    assert int(num_patches) == E * P

    shifts, use_eps = _host_ranges(Q, K, V, routes, betas, temp)
    plan = make_plan(routes, betas, temp, shifts, use_eps)
    nc = _get_nc(plan)
    in_maps = _stage_inputs(Q, K, V, plan[0])
    return nc, in_maps, plan


def kernel(Q_proj, K_proj, V_proj, betas, temperature, routes, num_patches):
    nc, in_maps, plan = prepare(Q_proj, K_proj, V_proj, betas, temperature,
                                routes, num_patches)
    res = run_bass_kernel_spmd(nc, in_maps, list(range(NCORES)))
    return _unstage_output(res.results, plan[0])
